# revision 1
# baseline (speedup 1.0000x reference)
"""Trainium2 Bass kernel for nn_Align: batched quaternion->rotmat + rigid transform.

reference math (per structure j of 64):
    q = (1, b, c, d) / sqrt(s),  s = 1 + b^2 + c^2 + d^2
    R = rotmat(q)                       # 3x3
    out[j] = pred[j] @ R + t[j]         # [91,3] @ [3,3] + [3]

Sharding: data-parallel over the 8 NeuronCores, 8 structures per core.

Per-core layout: partitions = (structure j:8, point-group g:13) = 104,
free dim = (point-in-group q:7, coord m:3) = 21.  R = N * (2/s) - I with
numerators N assembled from one broadcast-AP product op over the packed
row tail rc = [1 b c d b c]:  P[:, a+4b] = rc_a * rc_{a+b} gives
[1 bb cc dd | b bc cd db], so sum(P[0:4]) = s and the off-diagonal
products sit contiguously.  The transform runs as 9 fused
per-partition-scalar multiply-adds (3 per output coordinate).

Raw Bass (no Tile: this walrus build encodes at most one sync-wait per
compute instruction).  Every DVE RAW dep is semaphore-synced (streaming
same-engine RAW is not safe on HW), and the kernel clears its semaphores
then all-engine-barriers before use (sems persist across NEFF runs).
"""

import numpy as np

NCORES = 8
J = 8         # structures per core
G = 13        # point groups per structure
Q = 7         # points per group  (G*Q = 91)
PARTS = J * G  # 104 partitions

# R-tile column layout: [0:3]=diag(R00,R11,R22) [3:6]=plus(R10,R21,R02)
# [6:9]=minus(R01,R12,R20).  Columns holding (R[0,n], R[1,n], R[2,n]):
CHANNEL_COLS = {0: (0, 3, 8), 1: (6, 1, 4), 2: (5, 7, 2)}

_cache = {}


def _build_nc():
    import dataclasses

    import concourse.bass as bass
    import concourse.mybir as mybir

    f32 = mybir.dt.float32
    Alu = mybir.AluOpType

    nc = bass.Bass()
    # host-packed per (structure, point-group) row (30 floats):
    # [ 21 pred floats (7 points x 3 coords) | 1 b c d b c t0 t1 t2 ]
    packed = nc.dram_tensor("packed", [PARTS, 30], f32, kind="ExternalInput")
    out = nc.dram_tensor("out", [J, 91, 3], f32, kind="ExternalOutput")

    with (
        nc.sbuf_tensor([PARTS, 30], f32) as PK_t,
        nc.sbuf_tensor([PARTS, 8], f32) as P_t,
        nc.sbuf_tensor([PARTS, 9], f32) as R_t,
        nc.sbuf_tensor([PARTS, 1], f32) as S2_t,
        nc.sbuf_tensor([PARTS, 1], f32) as INV_t,
        nc.sbuf_tensor([PARTS, 6 * Q], f32) as ACC_t,
        nc.sbuf_tensor([PARTS, 21], f32) as O_t,
        nc.semaphore("dma_in") as dma_in_sem,
        nc.semaphore("v") as v_sem,
        nc.semaphore("dve_done") as dve_sem,
        nc.semaphore("dma_out") as dma_out_sem,
        nc.Block() as block,
    ):
        PK = PK_t[:, :]
        P = P_t[:, :]
        R = R_t[:, :]
        S2 = S2_t[:, :]
        INV = INV_t[:, :]
        O = O_t[:, :]
        ACC = [ACC_t[:, i * Q:(i + 1) * Q] for i in range(6)]
        RT = PK[:, 21:30]   # [1 b c d b c t0 t1 t2]

        def _pseudo_barrier(eng):
            # NRT expands this to a real all-engine barrier on runtime
            # semaphores outside the kernel sem range — stale-state proof.
            eng.isa(
                nc.isa.Opcode.NEURON_ISA_TPB_OPCODE_PSEUDO_SYNC_BARRIER,
                {},
                struct_name="NEURON_ISA_TPB_UNKNOWN_STRUCT",
                verify=False,
            )

        @block.gpsimd
        def _(gpsimd):
            # Stale-semaphore preamble: semaphores are NOT reset between NEFF
            # executions, and waits here use absolute values.  Clear every sem
            # this kernel waits on or increments, THEN barrier — without the
            # barrier an engine can pass its first wait on a stale value
            # before the clear lands (observed as a HW deadlock).  The Block
            # exit barrier's event sems (nc.barrier_sems) are self-managed
            # and were never cleared by the framework preamble either.
            nums = sorted(x.num for x in (dma_in_sem, v_sem, dve_sem, dma_out_sem))
            assert nums[-1] - nums[0] == 3, nums
            r = range(nums[0], nums[-1] + 1)
            gpsimd.dma_reset(r)
            gpsimd.sem_clear(r)
            _pseudo_barrier(gpsimd)

        @block.scalar
        def _(scalar):
            _pseudo_barrier(scalar)

        @block.tensor
        def _(tensor):
            _pseudo_barrier(tensor)

        @block.sync
        def _(sync):
            _pseudo_barrier(sync)
            sync.dma_start(out=PK, in_=packed[:, :]).then_inc(dma_in_sem, 16)
            sync.wait_ge(dve_sem, 1)
            sync.dma_start(
                out=out[:, :, :].rearrange("j (g q) m -> (j g) (q m)", g=G),
                in_=O,
            ).then_inc(dma_out_sem, 16)
            sync.wait_ge(dma_out_sem, 16)

        @block.vector
        def _(vector):
            _pseudo_barrier(vector)
            vector.wait_ge(dma_in_sem, 16)

            # DVE streaming RAW is not safe without sem sync (HW-verified):
            # every op bumps v_sem; consumers wait on the cumulative count.
            def op(k, *args, **kw):
                return getattr(vector, k)(*args, **kw).then_inc(v_sem, 1)

            # ---- rotation matrix ----
            # P[:, a+4b] = u_a * u_{a+b} over rc = RT[0:6] = [1 b c d b c],
            # b in {0,1}, a in {0,3}:
            #   b=0 -> [1 bb cc dd]   (cols 0:4; sum = s)
            #   b=1 -> [b bc cd db]   (cols 4:8; bc,cd,db at 5:8)
            u_ap = RT[:, 0:4].unsqueeze(1).broadcast_to([PARTS, 2, 4])
            v_base = RT[:, 0:4].unsqueeze(1).broadcast_to([PARTS, 2, 4])
            pairs = [list(p) for p in v_base.ap]
            pairs[1][0] = 1  # dims [partition, b, a]; b-step 1 elem -> u_{a+b}
            v_ap = dataclasses.replace(v_base, ap=pairs)
            p_out = P.rearrange("p (b a) -> p b a", b=2)
            op("tensor_tensor", out=p_out, in0=u_ap, in1=v_ap, op=Alu.mult)  # 1
            vector.wait_ge(v_sem, 1)
            op("reduce_sum", out=S2, in_=P[:, 0:4],                          # 2  s
               axis=mybir.AxisListType.X)
            vector.wait_ge(v_sem, 2)
            op("reciprocal", out=INV, in_=S2)                                # 3  1/s
            # numerators: diag = P[1:4]+1;  plus/minus = [bc,cd,db] -+ [d,b,c]
            op("tensor_scalar", out=R[:, 0:3], in0=P[:, 1:4], scalar1=1.0,   # 4
               scalar2=None, op0=Alu.add)
            op("tensor_tensor", out=R[:, 3:6], in0=P[:, 5:8],                # 5
               in1=RT[:, 3:6], op=Alu.add)
            op("tensor_tensor", out=R[:, 6:9], in0=P[:, 5:8],                # 6
               in1=RT[:, 3:6], op=Alu.subtract)
            vector.wait_ge(v_sem, 6)
            op("tensor_scalar", out=R, in0=R, scalar1=INV, scalar2=2.0,      # 7
               op0=Alu.mult, op1=Alu.mult)                                   #   R=num*2/s
            vector.wait_ge(v_sem, 7)
            op("tensor_scalar", out=R[:, 0:3], in0=R[:, 0:3], scalar1=-1.0,  # 8
               scalar2=None, op0=Alu.add)                                    #   diag -1

            # ---- transform (channel-interleaved) ----
            xm = PK[:, 0:21].rearrange("p (q m) -> p m q", m=3)
            om = O.rearrange("p (q m) -> p m q", m=3)
            a0 = [ACC[2 * n][:, :] for n in range(3)]
            a1 = [ACC[2 * n + 1][:, :] for n in range(3)]
            vector.wait_ge(v_sem, 8)
            for n in range(3):        # 9,10,11:  I1_n = X0*R[0,n] + t_n
                c0 = CHANNEL_COLS[n][0]
                op("tensor_scalar", out=a0[n], in0=xm[:, 0, :],
                   scalar1=R[:, c0:c0 + 1], scalar2=RT[:, 6 + n:7 + n],
                   op0=Alu.mult, op1=Alu.add)
            for n in range(3):        # 12,13,14:  I2_n = X1*R[1,n] + I1_n
                vector.wait_ge(v_sem, 9 + n)
                c1 = CHANNEL_COLS[n][1]
                op("scalar_tensor_tensor", out=a1[n], in0=xm[:, 1, :],
                   scalar=R[:, c1:c1 + 1], in1=a0[n],
                   op0=Alu.mult, op1=Alu.add)
            for n in range(3):        # 15,16,17:  out_n = X2*R[2,n] + I2_n
                vector.wait_ge(v_sem, 12 + n)
                c2 = CHANNEL_COLS[n][2]
                ins = vector.scalar_tensor_tensor(
                    out=om[:, n, :], in0=xm[:, 2, :],
                    scalar=R[:, c2:c2 + 1], in1=a1[n],
                    op0=Alu.mult, op1=Alu.add,
                )
                if n < 2:
                    ins.then_inc(v_sem, 1)
                else:
                    ins.then_inc(dve_sem, 1)

    return nc


def get_nc():
    if "nc" not in _cache:
        _cache["nc"] = _build_nc()
    return _cache["nc"]


def shard_inputs(pred_coor, r_vector, t_vector):
    # packed per (structure, group) row: [21 pred | 1 b c d b c | t0 t1 t2]
    n = pred_coor.shape[0]
    pk = np.empty((n, G, 30), dtype=np.float32)
    pk[:, :, 0:21] = pred_coor.reshape(n, G, 21)
    pk[:, :, 21] = 1.0
    pk[:, :, 22:25] = r_vector[:, None, :]
    pk[:, :, 25:27] = r_vector[:, None, 0:2]
    pk[:, :, 27:30] = t_vector[:, None, :]
    pk = pk.reshape(n * G, 30)
    return [
        {"packed": np.ascontiguousarray(pk[c * PARTS : (c + 1) * PARTS])}
        for c in range(NCORES)
    ]


def run(pred_coor, r_vector, t_vector, trace=False):
    from concourse.bass_utils import run_bass_kernel_spmd

    nc = get_nc()
    in_maps = shard_inputs(pred_coor, r_vector, t_vector)
    res = run_bass_kernel_spmd(nc, in_maps, list(range(NCORES)), trace=trace)
    full = np.concatenate([res.results[c]["out"] for c in range(NCORES)], axis=0)
    return full, res


def kernel(pred_coor, r_vector, t_vector):
    pred_coor = np.asarray(pred_coor, dtype=np.float32)
    r_vector = np.asarray(r_vector, dtype=np.float32)
    t_vector = np.asarray(t_vector, dtype=np.float32)
    full, _ = run(pred_coor, r_vector, t_vector, trace=False)
    return full



# revision 10
# speedup vs baseline: 1.5456x; 1.5456x over previous
"""Trainium2 Bass kernel for nn_Align: batched quaternion->rotmat + rigid transform.

reference math (per structure j of 64):
    q = (1, b, c, d) / sqrt(s),  s = 1 + b^2 + c^2 + d^2
    R = rotmat(q)                       # 3x3
    out[j] = pred[j] @ R + t[j]         # [91,3] @ [3,3] + [3]

Sharding: data-parallel over the 8 NeuronCores, 8 structures per core.

Per-core layout: partitions = (structure j:8, point-group g:13) = 104,
free dim = (point-in-group q:7, coord m:3) = 21.

Factorization: R = (2/s)*N - I with N = u (x) u + W, u = (b,c,d),
W = [[1,-d,c],[d,1,-b],[-c,b,1]] (host-packed signed copies), so

    out[q,n] = (2/s) * sum_m X[q,m]*N[m,n]  -  X[q,n]  +  t[n].

DVE chain (costs follow the TRN2 cost model: ~60ns fixed access + 1.04ns
per AP element, but any op whose APs are all single-element is free, and
plain tensor_scalar runs at 2x rate):
    P9  = u (x) u                  one broadcast-AP outer product [9]
    S3  = reduce(diag P9)          bb+cc+dd  (stride-4 AP)       [3]
    S2  = S3*0.5 + 0.5             = s/2                         [free]
    IV2 = 1/S2                     = 2/s                         [free]
    N9  = P9 + W9                                                [9]
    PA[q,n,m] = X[q,m]*N9[m,n]     one 3-free-dim broadcast TT   [63]
    ZN  = reduce_m(PA)             sum over innermost m          [63->21]
    O1  = (ZN * IV2) + t_bcast     scalar_tensor_tensor          [21]
    O   = O1 - X                   (the -I term; X is already in
                                    (q,n) layout)                [21]

Critical-path engineering (CoreSim cost model):
  - input DMA issued by the sync engine BEFORE the stale-semaphore barrier
    (its sem increment lands >1.3us after gpsimd's clears, so the
    clear-before-inc ordering holds with huge margin);
  - all cross-run-stale semaphores are cleared on gpsimd before the
    all-engine barrier; no dma_reset is needed (and with the pre-barrier
    input DMA it must not run: its drain would wait on the in-flight input
    DMA) because every DMA of a run completes before that run's engines
    drain, so no DGE state can leak across NEFF runs.
  (A semaphore-free output DMA would additionally hide the 900ns DMA-sem
  propagation tail, but walrus requires DGE sync info and a wait-only DGE
  hangs the device — HW-verified unrecoverable; keep full sem sync.)

Raw Bass (no Tile: this walrus build encodes at most one sync-wait per
compute instruction).  Every DVE RAW dep is semaphore-synced (streaming
same-engine RAW is not safe on HW).
"""

import dataclasses

import numpy as np

NCORES = 8
J = 8          # structures per core
G = 13         # point groups per structure
Q = 7          # points per group  (G*Q = 91)
PARTS = J * G  # 104 partitions

# packed row layout (36 floats per (j,g) row):
#   [0:21]  pred, (q,m) interleaved
#   [21:24] u3 = [b c d]
#   [24:33] W9 = [1 -d c  d 1 -b  -c b 1]   (row-major [m,n] addends)
#   [33:36] t
NPACK = 36
C_U3 = 21
C_W9 = 24
C_T = 33

_cache = {}


def _ap_stride(ap, dim, stride):
    """Return a copy of AP `ap` with dims[dim] stride replaced (elems)."""
    pairs = [list(p) for p in ap.ap]
    pairs[dim][0] = stride
    return dataclasses.replace(ap, ap=pairs)


def _build_nc():
    import concourse.bass as bass
    import concourse.mybir as mybir

    f32 = mybir.dt.float32
    Alu = mybir.AluOpType

    nc = bass.Bass()
    packed = nc.dram_tensor("packed", [PARTS, NPACK], f32, kind="ExternalInput")
    out = nc.dram_tensor("out", [J, 91, 3], f32, kind="ExternalOutput")

    with (
        nc.sbuf_tensor([PARTS, NPACK], f32) as PK_t,
        nc.sbuf_tensor([PARTS, 9], f32) as P9_t,
        nc.sbuf_tensor([PARTS, 1], f32) as S3_t,
        nc.sbuf_tensor([PARTS, 1], f32) as S2_t,
        nc.sbuf_tensor([PARTS, 1], f32) as IV2_t,
        nc.sbuf_tensor([PARTS, 9], f32) as N9_t,
        nc.sbuf_tensor([PARTS, 63], f32) as PA_t,
        nc.sbuf_tensor([PARTS, 21], f32) as ZN_t,
        nc.sbuf_tensor([PARTS, 21], f32) as O1_t,
        nc.sbuf_tensor([PARTS, 21], f32) as O_t,
        nc.semaphore("dma_in") as dma_in_sem,
        nc.semaphore("v") as v_sem,
        nc.semaphore("dve_done") as dve_sem,
        nc.semaphore("dma_out") as dma_out_sem,
        nc.Block() as block,
    ):
        PK = PK_t[:, :]
        P9 = P9_t[:, :]
        S3 = S3_t[:, :]
        S2 = S2_t[:, :]
        IV2 = IV2_t[:, :]
        N9 = N9_t[:, :]
        ZN = ZN_t[:, :]
        O1 = O1_t[:, :]
        O = O_t[:, :]
        X21 = PK[:, 0:21]

        def _pseudo_barrier(eng):
            # NRT expands this to a real all-engine barrier on runtime
            # semaphores outside the kernel sem range — stale-state proof.
            eng.isa(
                nc.isa.Opcode.NEURON_ISA_TPB_OPCODE_PSEUDO_SYNC_BARRIER,
                {},
                struct_name="NEURON_ISA_TPB_UNKNOWN_STRUCT",
                verify=False,
            )

        @block.gpsimd
        def _(gpsimd):
            # Stale-semaphore preamble: semaphores are NOT reset between NEFF
            # executions, and waits here use absolute values.  Clear every sem
            # this kernel waits on or increments, THEN barrier — without the
            # barrier an engine can pass its first wait on a stale value
            # before the clear lands (observed as a HW deadlock).
            nums = sorted(
                x.num for x in (dma_in_sem, v_sem, dve_sem, dma_out_sem)
            )
            assert nums[-1] - nums[0] == 3, nums
            gpsimd.sem_clear(range(nums[0], nums[-1] + 1))
            _pseudo_barrier(gpsimd)

        @block.scalar
        def _(scalar):
            _pseudo_barrier(scalar)

        @block.tensor
        def _(tensor):
            _pseudo_barrier(tensor)

        @block.sync
        def _(sync):
            # Input DMA ahead of the barrier: SP reads no semaphores, and the
            # DMA's sem increment can't land before its DGE delay + transfer
            # (~1.3us), far after gpsimd's t~100ns clears.
            sync.dma_start(out=PK, in_=packed[:, :]).then_inc(dma_in_sem, 16)
            _pseudo_barrier(sync)
            sync.wait_ge(dve_sem, 1)
            sync.dma_start(
                out=out[:, :, :].rearrange("j (g q) m -> (j g) (q m)", g=G),
                in_=O,
            ).then_inc(dma_out_sem, 16)
            sync.wait_ge(dma_out_sem, 16)

        @block.vector
        def _(vector):
            _pseudo_barrier(vector)
            vector.wait_ge(dma_in_sem, 16)

            # DVE streaming RAW is not safe without sem sync (HW-verified):
            # every op bumps v_sem; consumers wait on the cumulative count.
            def op(k, *args, **kw):
                return getattr(vector, k)(*args, **kw).then_inc(v_sem, 1)

            u3 = PK[:, C_U3:C_U3 + 3]
            op("tensor_tensor",                                              # 1
               out=P9.rearrange("p (m n) -> p m n", n=3),
               in0=u3.unsqueeze(2).broadcast_to([PARTS, 3, 3]),
               in1=u3.unsqueeze(1).broadcast_to([PARTS, 3, 3]),
               op=Alu.mult)                                    # P9[m,n]=u_m*u_n
            vector.wait_ge(v_sem, 1)
            op("reduce_sum", out=S3,                                         # 2
               in_=_ap_stride(P9_t[:, 0:3], 1, 4),             # diag: bb+cc+dd
               axis=mybir.AxisListType.X)
            vector.wait_ge(v_sem, 2)
            op("tensor_scalar", out=S2, in0=S3, scalar1=0.5, scalar2=0.5,    # 3
               op0=Alu.mult, op1=Alu.add)                      # s/2   (free)
            vector.wait_ge(v_sem, 3)
            op("reciprocal", out=IV2, in_=S2)                  # 2/s   (free) # 4
            op("tensor_tensor", out=N9, in0=P9,                              # 5
               in1=PK[:, C_W9:C_W9 + 9], op=Alu.add)           # N = P9 + W9
            vector.wait_ge(v_sem, 5)
            # PA[q,n,m] = X[q,m] * N9[m,n]; one op, 3 broadcast free dims.
            op("tensor_tensor",                                              # 6
               out=PA_t[:, :].rearrange("p (q n m) -> p q n m", n=3, m=3),
               in0=PK_t[:, 0:21].rearrange("p (q m) -> p q m", m=3)
                   .unsqueeze(2).broadcast_to([PARTS, 7, 3, 3]),
                                                               # X: (q s3,n s0,m s1)
               in1=N9_t[:, 0:9].rearrange("p (m n) -> p n m", n=3)
                   .unsqueeze(1).broadcast_to([PARTS, 7, 3, 3]),
                                                               # N9: (q s0,n s1,m s3)
               op=Alu.mult)
            vector.wait_ge(v_sem, 6)
            op("reduce_sum", out=ZN,                                         # 7
               in_=PA_t[:, :].rearrange("p (q n m) -> p q n m", n=3, m=3),
               axis=mybir.AxisListType.X)                      # sum over m
            vector.wait_ge(v_sem, 7)
            op("scalar_tensor_tensor",                                       # 8
               out=O1_t[:, :].rearrange("p (q n) -> p q n", n=3),
               in0=ZN_t[:, :].rearrange("p (q n) -> p q n", n=3),
               scalar=IV2,
               in1=PK[:, C_T:C_T + 3].unsqueeze(1).broadcast_to([PARTS, 7, 3]),
               op0=Alu.mult, op1=Alu.add)                      # ZN*2/s + t
            vector.wait_ge(v_sem, 8)
            vector.tensor_tensor(                                            # 9
                out=O, in0=O1, in1=X21, op=Alu.subtract,       # - X  (the -I)
            ).then_inc(dve_sem, 1)

    return nc


def get_nc():
    if "nc" not in _cache:
        _cache["nc"] = _build_nc()
    return _cache["nc"]


def shard_inputs(pred_coor, r_vector, t_vector):
    n = pred_coor.shape[0]
    b, c, d = r_vector[:, 0], r_vector[:, 1], r_vector[:, 2]
    one = np.ones_like(b)
    w9 = np.stack([one, -d, c, d, one, -b, -c, b, one], axis=-1)  # [n,9]
    pk = np.empty((n, G, NPACK), dtype=np.float32)
    pk[:, :, 0:21] = pred_coor.reshape(n, G, 21)
    pk[:, :, C_U3:C_U3 + 3] = r_vector[:, None, :]
    pk[:, :, C_W9:C_W9 + 9] = w9[:, None, :]
    pk[:, :, C_T:C_T + 3] = t_vector[:, None, :]
    pk = pk.reshape(n * G, NPACK)
    return [
        {"packed": np.ascontiguousarray(pk[c * PARTS : (c + 1) * PARTS])}
        for c in range(NCORES)
    ]


def run(pred_coor, r_vector, t_vector, trace=False):
    from concourse.bass_utils import run_bass_kernel_spmd

    nc = get_nc()
    in_maps = shard_inputs(pred_coor, r_vector, t_vector)
    res = run_bass_kernel_spmd(nc, in_maps, list(range(NCORES)), trace=trace)
    full = np.concatenate([res.results[c]["out"] for c in range(NCORES)], axis=0)
    return full, res


def kernel(pred_coor, r_vector, t_vector):
    pred_coor = np.asarray(pred_coor, dtype=np.float32)
    r_vector = np.asarray(r_vector, dtype=np.float32)
    t_vector = np.asarray(t_vector, dtype=np.float32)
    full, _ = run(pred_coor, r_vector, t_vector, trace=False)
    return full


# revision 11
# speedup vs baseline: 1.6667x; 1.0783x over previous
"""Trainium2 Bass kernel for nn_Align: batched quaternion->rotmat + rigid transform.

reference math (per structure j of 64):
    q = (1, b, c, d) / sqrt(s),  s = 1 + b^2 + c^2 + d^2
    R = rotmat(q)                       # 3x3
    out[j] = pred[j] @ R + t[j]         # [91,3] @ [3,3] + [3]

Sharding: data-parallel over the 8 NeuronCores, 8 structures per core.

Per-core layout: partitions = (structure j:8, point-group g:13) = 104,
free dim = (point-in-group q:7, coord m:3) = 21.

Factorization: R = (2/s)*N - I with N = u (x) u + W, u = (b,c,d),
W = [[1,-d,c],[d,1,-b],[-c,b,1]] (host-packed signed copies), so

    out[q,n] = (2/s) * sum_m X[q,m]*N[m,n]  +  (t[n] - X[q,n]).

Engine split:
  DVE:  per-partition scalar pipeline (all APs single-element, so each op
        streams at near-zero marginal cost on the TRN2 DVE):
          N9[3m+n] = u_m*u_n + W[m,n]     9x scalar_tensor_tensor
          S2 = ((b*b/2+.5) + c*c/2) + d*d/2 = s/2   (h = u/2 host-packed)
          IV2 = 1/S2 = 2/s
        then the per-point work:
          PA[q,n,m] = X[q,m]*N9[m,n]      one 3-free-dim broadcast TT [63]
          ZN = reduce_m(PA)               innermost-axis reduce  [63->21]
          O  = (ZN * IV2) + TXm           scalar_tensor_tensor   [21]
  Pool (gpsimd): TXm = t_bcast - X  [21], overlapped with the DVE pipeline
        (it only needs the input DMA).

Critical-path engineering (CoreSim cost model):
  - input DMA issued by the sync engine BEFORE the stale-semaphore barrier
    (its sem increment lands >1.3us after gpsimd's clears, so the
    clear-before-inc ordering holds with huge margin);
  - all cross-run-stale semaphores are cleared on gpsimd before the
    all-engine barrier; no dma_reset is needed (and with the pre-barrier
    input DMA it must not run: its drain would wait on the in-flight input
    DMA) because every DMA of a run completes before that run's engines
    drain, so no DGE state can leak across NEFF runs.
  (A semaphore-free output DMA would additionally hide the 900ns DMA-sem
  propagation tail, but walrus requires DGE sync info and a wait-only DGE
  hangs the device — HW-verified unrecoverable; keep full sem sync.)

Raw Bass (no Tile: this walrus build encodes at most one sync-wait per
compute instruction).  Every cross-op RAW dep is semaphore-synced
(streaming same-engine RAW is not safe on HW).
"""

import numpy as np

NCORES = 8
J = 8          # structures per core
G = 13         # point groups per structure
Q = 7          # points per group  (G*Q = 91)
PARTS = J * G  # 104 partitions

# packed row layout (39 floats per (j,g) row):
#   [0:21]  pred, (q,m) interleaved
#   [21:24] u3 = [b c d]
#   [24:33] W9 = [1 -d c  d 1 -b  -c b 1]   (row-major [m,n] addends)
#   [33:36] t
#   [36:39] h3 = u3/2
NPACK = 39
C_U3 = 21
C_W9 = 24
C_T = 33
C_H3 = 36

_cache = {}


def _build_nc():
    import concourse.bass as bass
    import concourse.mybir as mybir

    f32 = mybir.dt.float32
    Alu = mybir.AluOpType

    nc = bass.Bass()
    packed = nc.dram_tensor("packed", [PARTS, NPACK], f32, kind="ExternalInput")
    out = nc.dram_tensor("out", [J, 91, 3], f32, kind="ExternalOutput")

    with (
        nc.sbuf_tensor([PARTS, NPACK], f32) as PK_t,
        nc.sbuf_tensor([PARTS, 9], f32) as N9_t,
        nc.sbuf_tensor([PARTS, 1], f32) as A_t,
        nc.sbuf_tensor([PARTS, 1], f32) as B_t,
        nc.sbuf_tensor([PARTS, 1], f32) as S2_t,
        nc.sbuf_tensor([PARTS, 1], f32) as IV2_t,
        nc.sbuf_tensor([PARTS, 63], f32) as PA_t,
        nc.sbuf_tensor([PARTS, 21], f32) as ZN_t,
        nc.sbuf_tensor([PARTS, 21], f32) as TX_t,
        nc.sbuf_tensor([PARTS, 21], f32) as O_t,
        nc.semaphore("dma_in") as dma_in_sem,
        nc.semaphore("v") as v_sem,
        nc.semaphore("dve_done") as dve_sem,
        nc.semaphore("dma_out") as dma_out_sem,
        nc.semaphore("txm") as txm_sem,
        nc.Block() as block,
    ):
        PK = PK_t[:, :]
        O = O_t[:, :]
        X21 = PK[:, 0:21]

        def ucol(m):
            return PK[:, C_U3 + m:C_U3 + m + 1]

        def hcol(m):
            return PK[:, C_H3 + m:C_H3 + m + 1]

        def _pseudo_barrier(eng):
            # NRT expands this to a real all-engine barrier on runtime
            # semaphores outside the kernel sem range — stale-state proof.
            eng.isa(
                nc.isa.Opcode.NEURON_ISA_TPB_OPCODE_PSEUDO_SYNC_BARRIER,
                {},
                struct_name="NEURON_ISA_TPB_UNKNOWN_STRUCT",
                verify=False,
            )

        @block.gpsimd
        def _(gpsimd):
            # Stale-semaphore preamble: semaphores are NOT reset between NEFF
            # executions, and waits here use absolute values.  Clear every sem
            # this kernel waits on or increments, THEN barrier — without the
            # barrier an engine can pass its first wait on a stale value
            # before the clear lands (observed as a HW deadlock).
            nums = sorted(
                x.num
                for x in (dma_in_sem, v_sem, dve_sem, dma_out_sem, txm_sem)
            )
            assert nums[-1] - nums[0] == 4, nums
            gpsimd.sem_clear(range(nums[0], nums[-1] + 1))
            _pseudo_barrier(gpsimd)
            # TXm = t - X, the R-independent part of the output, overlapped
            # with the DVE pipeline.
            gpsimd.wait_ge(dma_in_sem, 16)
            gpsimd.tensor_tensor(
                out=TX_t[:, :].rearrange("p (q n) -> p q n", n=3),
                in0=PK[:, C_T:C_T + 3].unsqueeze(1).broadcast_to([PARTS, 7, 3]),
                in1=X21.rearrange("p (q n) -> p q n", n=3),
                op=Alu.subtract,
            ).then_inc(txm_sem, 1)

        @block.scalar
        def _(scalar):
            _pseudo_barrier(scalar)

        @block.tensor
        def _(tensor):
            _pseudo_barrier(tensor)

        @block.sync
        def _(sync):
            # Input DMA ahead of the barrier: SP reads no semaphores, and the
            # DMA's sem increment can't land before its DGE delay + transfer
            # (~1.3us), far after gpsimd's t~100ns clears.
            sync.dma_start(out=PK, in_=packed[:, :]).then_inc(dma_in_sem, 16)
            _pseudo_barrier(sync)
            sync.wait_ge(dve_sem, 1)
            sync.dma_start(
                out=out[:, :, :].rearrange("j (g q) m -> (j g) (q m)", g=G),
                in_=O,
            ).then_inc(dma_out_sem, 16)
            sync.wait_ge(dma_out_sem, 16)

        @block.vector
        def _(vector):
            _pseudo_barrier(vector)
            vector.wait_ge(dma_in_sem, 16)

            # Every cross-op RAW dep is sem-synced: each op bumps v_sem,
            # consumers wait on the producer's cumulative count.
            def op(k, *args, **kw):
                return getattr(vector, k)(*args, **kw).then_inc(v_sem, 1)

            # ---- R numerators, one scalar_tensor_tensor per element ----
            # ops 1..9:  N9[3m+n] = u_m * u_n + W[m,n]
            for m in range(3):
                for n in range(3):
                    k = 3 * m + n
                    op("scalar_tensor_tensor", out=N9_t[:, k:k + 1],
                       in0=ucol(m), scalar=ucol(n),
                       in1=PK[:, C_W9 + k:C_W9 + k + 1],
                       op0=Alu.mult, op1=Alu.add)
            # ops 10..13: s/2 then 2/s, all single-element (near-free)
            op("tensor_scalar", out=A_t[:, :], in0=ucol(0), scalar1=hcol(0),  # 10
               scalar2=0.5, op0=Alu.mult, op1=Alu.add)         # bb/2 + 1/2
            vector.wait_ge(v_sem, 10)
            op("scalar_tensor_tensor", out=B_t[:, :], in0=ucol(1),           # 11
               scalar=hcol(1), in1=A_t[:, :], op0=Alu.mult, op1=Alu.add)
            vector.wait_ge(v_sem, 11)
            op("scalar_tensor_tensor", out=S2_t[:, :], in0=ucol(2),          # 12
               scalar=hcol(2), in1=B_t[:, :], op0=Alu.mult, op1=Alu.add)
            vector.wait_ge(v_sem, 12)
            op("reciprocal", out=IV2_t[:, :], in_=S2_t[:, :])  # 2/s         # 13
            # ---- per-point work ----
            vector.wait_ge(v_sem, 9)
            # PA[q,n,m] = X[q,m] * N9[m,n]; one op, 3 broadcast free dims.
            op("tensor_tensor",                                              # 14
               out=PA_t[:, :].rearrange("p (q n m) -> p q n m", n=3, m=3),
               in0=PK_t[:, 0:21].rearrange("p (q m) -> p q m", m=3)
                   .unsqueeze(2).broadcast_to([PARTS, 7, 3, 3]),
                                                               # X: (q s3,n s0,m s1)
               in1=N9_t[:, 0:9].rearrange("p (m n) -> p n m", n=3)
                   .unsqueeze(1).broadcast_to([PARTS, 7, 3, 3]),
                                                               # N9: (q s0,n s1,m s3)
               op=Alu.mult)
            vector.wait_ge(v_sem, 14)
            op("reduce_sum", out=ZN_t[:, :],                                 # 15
               in_=PA_t[:, :].rearrange("p (q n m) -> p q n m", n=3, m=3),
               axis=mybir.AxisListType.X)                      # sum over m
            vector.wait_ge(v_sem, 15)
            vector.wait_ge(txm_sem, 1)
            vector.scalar_tensor_tensor(                                     # 16
                out=O, in0=ZN_t[:, :], scalar=IV2_t[:, :], in1=TX_t[:, :],
                op0=Alu.mult, op1=Alu.add,                     # ZN*2/s + (t-X)
            ).then_inc(dve_sem, 1)

    return nc


def get_nc():
    if "nc" not in _cache:
        _cache["nc"] = _build_nc()
    return _cache["nc"]


def shard_inputs(pred_coor, r_vector, t_vector):
    n = pred_coor.shape[0]
    b, c, d = r_vector[:, 0], r_vector[:, 1], r_vector[:, 2]
    one = np.ones_like(b)
    w9 = np.stack([one, -d, c, d, one, -b, -c, b, one], axis=-1)  # [n,9]
    pk = np.empty((n, G, NPACK), dtype=np.float32)
    pk[:, :, 0:21] = pred_coor.reshape(n, G, 21)
    pk[:, :, C_U3:C_U3 + 3] = r_vector[:, None, :]
    pk[:, :, C_W9:C_W9 + 9] = w9[:, None, :]
    pk[:, :, C_T:C_T + 3] = t_vector[:, None, :]
    pk[:, :, C_H3:C_H3 + 3] = 0.5 * r_vector[:, None, :]
    pk = pk.reshape(n * G, NPACK)
    return [
        {"packed": np.ascontiguousarray(pk[c * PARTS : (c + 1) * PARTS])}
        for c in range(NCORES)
    ]


def run(pred_coor, r_vector, t_vector, trace=False):
    from concourse.bass_utils import run_bass_kernel_spmd

    nc = get_nc()
    in_maps = shard_inputs(pred_coor, r_vector, t_vector)
    res = run_bass_kernel_spmd(nc, in_maps, list(range(NCORES)), trace=trace)
    full = np.concatenate([res.results[c]["out"] for c in range(NCORES)], axis=0)
    return full, res


def kernel(pred_coor, r_vector, t_vector):
    pred_coor = np.asarray(pred_coor, dtype=np.float32)
    r_vector = np.asarray(r_vector, dtype=np.float32)
    t_vector = np.asarray(t_vector, dtype=np.float32)
    full, _ = run(pred_coor, r_vector, t_vector, trace=False)
    return full


# revision 14
# speedup vs baseline: 1.7136x; 1.0282x over previous
"""Trainium2 Bass kernel for nn_Align: batched quaternion->rotmat + rigid transform.

reference math (per structure j of 64):
    q = (1, b, c, d) / sqrt(s),  s = 1 + b^2 + c^2 + d^2
    R = rotmat(q)                       # 3x3
    out[j] = pred[j] @ R + t[j]         # [91,3] @ [3,3] + [3]

Sharding: data-parallel over the 8 NeuronCores, 8 structures per core.

Per-core layout: partitions = (structure j:8, point-group g:13) = 104,
free dim = (point-in-group q:7, coord m:3) = 21.

Factorization: R = (2/s)*N - I with N = u (x) u + W, u = (b,c,d),
W = [[1,-d,c],[d,1,-b],[-c,b,1]] (host-packed signed copies), so

    out[q,n] = (2/s) * sum_m X[q,m]*N[m,n]  +  (t[n] - X[q,n]).

Engine split:
  DVE:  per-partition scalar pipeline (all APs single-element, so each op
        streams at near-zero marginal cost on the TRN2 DVE):
          N9[3m+n] = u_m*u_n + W[m,n]     9x scalar_tensor_tensor
          S2 = ((b*b/2+.5) + c*c/2) + d*d/2 = s/2   (h = u/2 host-packed)
          IV2 = 1/S2 = 2/s
        then the per-point work:
          PA[q,n,m] = X[q,m]*N9[m,n]      one 3-free-dim broadcast TT [63]
          ZN = reduce_m(PA)               innermost-axis reduce  [63->21]
          O  = (ZN * IV2) + TXm           scalar_tensor_tensor   [21]
  Pool (gpsimd): TXm = t_bcast - X  [21], overlapped with the DVE pipeline
        (it only needs the input DMA).

Critical-path engineering (CoreSim cost model):
  - input DMA issued on gpsimd right after its semaphore clears and BEFORE
    the stale-semaphore barrier (gpsimd's first instruction slot is ~100ns
    earlier than the sync engine's, and the DMA's sem increment lands
    >1.3us after the clears, so clear-before-inc ordering holds with huge
    margin);
  - all cross-run-stale semaphores are cleared on gpsimd before the
    all-engine barrier; no dma_reset is needed (and with the pre-barrier
    input DMA it must not run: its drain would wait on the in-flight input
    DMA) because every DMA of a run completes before that run's engines
    drain, so no DGE state can leak across NEFF runs;
  - output DMA on the sync engine, gated by one dve_done semaphore hop.
  (A semaphore-free output DMA would additionally hide the DMA-sem
  propagation tail, but walrus requires DGE sync info and a wait-only DGE
  hangs the device — HW-verified unrecoverable; keep full sem sync.)

Raw Bass (no Tile: this walrus build encodes at most one sync-wait per
compute instruction).  Every cross-op RAW dep is semaphore-synced
(streaming same-engine RAW is not safe on HW).
"""

import numpy as np

NCORES = 8
J = 8          # structures per core
G = 13         # point groups per structure
Q = 7          # points per group  (G*Q = 91)
PARTS = J * G  # 104 partitions

# packed row layout (39 floats per (j,g) row):
#   [0:21]  pred, (q,m) interleaved
#   [21:24] u3 = [b c d]
#   [24:33] W9 = [1 -d c  d 1 -b  -c b 1]   (row-major [m,n] addends)
#   [33:36] t
#   [36:39] h3 = u3/2
NPACK = 39
C_U3 = 21
C_W9 = 24
C_T = 33
C_H3 = 36

_cache = {}


def _build_nc():
    import concourse.bass as bass
    import concourse.mybir as mybir

    f32 = mybir.dt.float32
    Alu = mybir.AluOpType

    nc = bass.Bass()
    packed = nc.dram_tensor("packed", [PARTS, NPACK], f32, kind="ExternalInput")
    out = nc.dram_tensor("out", [J, 91, 3], f32, kind="ExternalOutput")

    with (
        nc.sbuf_tensor([PARTS, NPACK], f32) as PK_t,
        nc.sbuf_tensor([PARTS, 9], f32) as N9_t,
        nc.sbuf_tensor([PARTS, 1], f32) as A_t,
        nc.sbuf_tensor([PARTS, 1], f32) as B_t,
        nc.sbuf_tensor([PARTS, 1], f32) as S2_t,
        nc.sbuf_tensor([PARTS, 1], f32) as IV2_t,
        nc.sbuf_tensor([PARTS, 63], f32) as PA_t,
        nc.sbuf_tensor([PARTS, 21], f32) as ZN_t,
        nc.sbuf_tensor([PARTS, 21], f32) as TX_t,
        nc.sbuf_tensor([PARTS, 21], f32) as O_t,
        nc.semaphore("dma_in") as dma_in_sem,
        nc.semaphore("v") as v_sem,
        nc.semaphore("dve_done") as dve_sem,
        nc.semaphore("dma_out") as dma_out_sem,
        nc.semaphore("txm") as txm_sem,
        nc.Block() as block,
    ):
        PK = PK_t[:, :]
        O = O_t[:, :]
        X21 = PK[:, 0:21]

        def ucol(m):
            return PK[:, C_U3 + m:C_U3 + m + 1]

        def hcol(m):
            return PK[:, C_H3 + m:C_H3 + m + 1]

        def _pseudo_barrier(eng):
            # NRT expands this to a real all-engine barrier on runtime
            # semaphores outside the kernel sem range — stale-state proof.
            eng.isa(
                nc.isa.Opcode.NEURON_ISA_TPB_OPCODE_PSEUDO_SYNC_BARRIER,
                {},
                struct_name="NEURON_ISA_TPB_UNKNOWN_STRUCT",
                verify=False,
            )

        @block.gpsimd
        def _(gpsimd):
            # Stale-semaphore preamble: semaphores are NOT reset between NEFF
            # executions, and waits here use absolute values.  Clear every sem
            # this kernel waits on or increments, THEN barrier — without the
            # barrier an engine can pass its first wait on a stale value
            # before the clear lands (observed as a HW deadlock).
            nums = sorted(
                x.num
                for x in (dma_in_sem, v_sem, dve_sem, dma_out_sem, txm_sem)
            )
            assert nums[-1] - nums[0] == 4, nums
            gpsimd.sem_clear(range(nums[0], nums[-1] + 1))
            gpsimd.dma_start(out=PK, in_=packed[:, :]).then_inc(dma_in_sem, 16)
            _pseudo_barrier(gpsimd)
            # TXm = t - X, the R-independent part of the output, overlapped
            # with the DVE pipeline.
            gpsimd.wait_ge(dma_in_sem, 16)
            gpsimd.tensor_tensor(
                out=TX_t[:, :].rearrange("p (q n) -> p q n", n=3),
                in0=PK[:, C_T:C_T + 3].unsqueeze(1).broadcast_to([PARTS, 7, 3]),
                in1=X21.rearrange("p (q n) -> p q n", n=3),
                op=Alu.subtract,
            ).then_inc(txm_sem, 1)

        @block.scalar
        def _(scalar):
            _pseudo_barrier(scalar)

        @block.tensor
        def _(tensor):
            _pseudo_barrier(tensor)

        @block.sync
        def _(sync):
            _pseudo_barrier(sync)
            sync.wait_ge(dve_sem, 1)
            sync.dma_start(
                out=out[:, :, :].rearrange("j (g q) m -> (j g) (q m)", g=G),
                in_=O,
            ).then_inc(dma_out_sem, 16)
            sync.wait_ge(dma_out_sem, 16)

        @block.vector
        def _(vector):
            _pseudo_barrier(vector)
            vector.wait_ge(dma_in_sem, 16)

            # Every cross-op RAW dep is sem-synced: each op bumps v_sem,
            # consumers wait on the producer's cumulative count.
            def op(k, *args, **kw):
                return getattr(vector, k)(*args, **kw).then_inc(v_sem, 1)

            # ---- R numerators, one scalar_tensor_tensor per element ----
            # ops 1..9:  N9[3m+n] = u_m * u_n + W[m,n]
            for m in range(3):
                for n in range(3):
                    k = 3 * m + n
                    op("scalar_tensor_tensor", out=N9_t[:, k:k + 1],
                       in0=ucol(m), scalar=ucol(n),
                       in1=PK[:, C_W9 + k:C_W9 + k + 1],
                       op0=Alu.mult, op1=Alu.add)
            # ops 10..13: s/2 then 2/s, all single-element (near-free)
            op("tensor_scalar", out=A_t[:, :], in0=ucol(0), scalar1=hcol(0),  # 10
               scalar2=0.5, op0=Alu.mult, op1=Alu.add)         # bb/2 + 1/2
            vector.wait_ge(v_sem, 10)
            op("scalar_tensor_tensor", out=B_t[:, :], in0=ucol(1),           # 11
               scalar=hcol(1), in1=A_t[:, :], op0=Alu.mult, op1=Alu.add)
            vector.wait_ge(v_sem, 11)
            op("scalar_tensor_tensor", out=S2_t[:, :], in0=ucol(2),          # 12
               scalar=hcol(2), in1=B_t[:, :], op0=Alu.mult, op1=Alu.add)
            vector.wait_ge(v_sem, 12)
            op("reciprocal", out=IV2_t[:, :], in_=S2_t[:, :])  # 2/s         # 13
            # ---- per-point work ----
            vector.wait_ge(v_sem, 9)
            # PA[q,n,m] = X[q,m] * N9[m,n]; one op, 3 broadcast free dims.
            op("tensor_tensor",                                              # 14
               out=PA_t[:, :].rearrange("p (q n m) -> p q n m", n=3, m=3),
               in0=PK_t[:, 0:21].rearrange("p (q m) -> p q m", m=3)
                   .unsqueeze(2).broadcast_to([PARTS, 7, 3, 3]),
                                                               # X: (q s3,n s0,m s1)
               in1=N9_t[:, 0:9].rearrange("p (m n) -> p n m", n=3)
                   .unsqueeze(1).broadcast_to([PARTS, 7, 3, 3]),
                                                               # N9: (q s0,n s1,m s3)
               op=Alu.mult)
            vector.wait_ge(v_sem, 14)
            op("reduce_sum", out=ZN_t[:, :],                                 # 15
               in_=PA_t[:, :].rearrange("p (q n m) -> p q n m", n=3, m=3),
               axis=mybir.AxisListType.X)                      # sum over m
            vector.wait_ge(v_sem, 15)
            vector.wait_ge(txm_sem, 1)
            vector.scalar_tensor_tensor(                                     # 16
                out=O, in0=ZN_t[:, :], scalar=IV2_t[:, :], in1=TX_t[:, :],
                op0=Alu.mult, op1=Alu.add,                     # ZN*2/s + (t-X)
            ).then_inc(dve_sem, 1)

    return nc


def get_nc():
    if "nc" not in _cache:
        _cache["nc"] = _build_nc()
    return _cache["nc"]


def shard_inputs(pred_coor, r_vector, t_vector):
    n = pred_coor.shape[0]
    b, c, d = r_vector[:, 0], r_vector[:, 1], r_vector[:, 2]
    one = np.ones_like(b)
    w9 = np.stack([one, -d, c, d, one, -b, -c, b, one], axis=-1)  # [n,9]
    pk = np.empty((n, G, NPACK), dtype=np.float32)
    pk[:, :, 0:21] = pred_coor.reshape(n, G, 21)
    pk[:, :, C_U3:C_U3 + 3] = r_vector[:, None, :]
    pk[:, :, C_W9:C_W9 + 9] = w9[:, None, :]
    pk[:, :, C_T:C_T + 3] = t_vector[:, None, :]
    pk[:, :, C_H3:C_H3 + 3] = 0.5 * r_vector[:, None, :]
    pk = pk.reshape(n * G, NPACK)
    return [
        {"packed": np.ascontiguousarray(pk[c * PARTS : (c + 1) * PARTS])}
        for c in range(NCORES)
    ]


def run(pred_coor, r_vector, t_vector, trace=False):
    from concourse.bass_utils import run_bass_kernel_spmd

    nc = get_nc()
    in_maps = shard_inputs(pred_coor, r_vector, t_vector)
    res = run_bass_kernel_spmd(nc, in_maps, list(range(NCORES)), trace=trace)
    full = np.concatenate([res.results[c]["out"] for c in range(NCORES)], axis=0)
    return full, res


def kernel(pred_coor, r_vector, t_vector):
    pred_coor = np.asarray(pred_coor, dtype=np.float32)
    r_vector = np.asarray(r_vector, dtype=np.float32)
    t_vector = np.asarray(t_vector, dtype=np.float32)
    full, _ = run(pred_coor, r_vector, t_vector, trace=False)
    return full


# revision 15
# speedup vs baseline: 1.8822x; 1.0984x over previous
"""Trainium2 Bass kernel for nn_Align: batched quaternion->rotmat + rigid transform.

reference math (per structure j of 64):
    q = (1, b, c, d) / sqrt(s),  s = 1 + b^2 + c^2 + d^2
    R = rotmat(q)                       # 3x3
    out[j] = pred[j] @ R + t[j]         # [91,3] @ [3,3] + [3]

Sharding: data-parallel over the 8 NeuronCores, 8 structures per core.

Per-core layout: partitions = (structure j:8, point-group g:13) = 104,
free dim = (point-in-group q:7, coord m:3) = 21.

Factorization: R = (2/s)*N - I with N = u (x) u + W, u = (b,c,d),
W = [[1,-d,c],[d,1,-b],[-c,b,1]] (host-packed signed copies), so

    out[q,n] = (2/s) * sum_m X[q,m]*N[m,n]  +  (t[n] - X[q,n]).

Engine split:
  DVE:  per-partition scalar pipeline (all APs single-element, so each op
        streams at near-zero marginal cost on the TRN2 DVE):
          N9[3m+n] = u_m*u_n + W[m,n]     9x scalar_tensor_tensor
          S2 = ((b*b/2+.5) + c*c/2) + d*d/2 = s/2   (h = u/2 host-packed)
          IV2 = 1/S2 = 2/s
        then the per-point work:
          PA[q,n,m] = X[q,m]*N9[m,n]      one 3-free-dim broadcast TT [63]
          ZN = reduce_m(PA)               innermost-axis reduce  [63->21]
          O  = (ZN * IV2) + TXm           scalar_tensor_tensor   [21]
  Pool (gpsimd): TXm = t_bcast - X  [21], overlapped with the DVE pipeline
        (it only needs the input DMA).

Critical-path engineering (CoreSim cost model):
  - input DMA issued on gpsimd right after its semaphore clears and BEFORE
    the stale-semaphore barrier (gpsimd's first instruction slot is ~100ns
    earlier than the sync engine's, and the DMA's sem increment lands
    >1.3us after the clears, so clear-before-inc ordering holds with huge
    margin);
  - all cross-run-stale semaphores are cleared on gpsimd before the
    all-engine barrier; no dma_reset is needed (and with the pre-barrier
    input DMA it must not run: its drain would wait on the in-flight input
    DMA) because every DMA of a run completes before that run's engines
    drain, so no DGE state can leak across NEFF runs;
  - output DMA on the sync engine, gated by one dve_done semaphore hop.
  (A semaphore-free output DMA would additionally hide the DMA-sem
  propagation tail, but walrus requires DGE sync info and a wait-only DGE
  hangs the device — HW-verified unrecoverable; keep full sem sync.)

Raw Bass (no Tile: this walrus build encodes at most one sync-wait per
compute instruction).  Every cross-op RAW dep is semaphore-synced
(streaming same-engine RAW is not safe on HW).
"""

import numpy as np

NCORES = 8
J = 8          # structures per core
G = 13         # point groups per structure
Q = 7          # points per group  (G*Q = 91)
PARTS = J * G  # 104 partitions

# packed row layout (39 floats per (j,g) row):
#   [0:21]  pred, (q,m) interleaved
#   [21:24] u3 = [b c d]
#   [24:33] W9 = [1 -d c  d 1 -b  -c b 1]   (row-major [m,n] addends)
#   [33:36] t
#   [36:39] h3 = u3/2
NPACK = 39
GROWS = 112     # gather rows (104 data + 8 padding, multiple of 16)
GCOLS = 64      # gather row floats (256B elem_size granularity)
C_U3 = 21
C_W9 = 24
C_T = 33
C_H3 = 36

_cache = {}


def _build_nc():
    import concourse.bass as bass
    import concourse.mybir as mybir

    f32 = mybir.dt.float32
    Alu = mybir.AluOpType

    i16 = mybir.dt.int16

    nc = bass.Bass()
    packed = nc.dram_tensor("packed", [GROWS, GCOLS], f32, kind="ExternalInput")
    out = nc.dram_tensor("out", [J, 91, 3], f32, kind="ExternalOutput")

    with (
        nc.sbuf_tensor([128, GCOLS], f32) as PK_t,
        nc.sbuf_tensor([128, 8], i16) as IDX_t,
        nc.sbuf_tensor([PARTS, 9], f32) as N9_t,
        nc.sbuf_tensor([PARTS, 1], f32) as A_t,
        nc.sbuf_tensor([PARTS, 1], f32) as B_t,
        nc.sbuf_tensor([PARTS, 1], f32) as S2_t,
        nc.sbuf_tensor([PARTS, 1], f32) as IV2_t,
        nc.sbuf_tensor([PARTS, 63], f32) as PA_t,
        nc.sbuf_tensor([PARTS, 21], f32) as ZN_t,
        nc.sbuf_tensor([PARTS, 21], f32) as TX_t,
        nc.sbuf_tensor([PARTS, 21], f32) as O_t,
        nc.semaphore("dma_in") as dma_in_sem,
        nc.semaphore("v") as v_sem,
        nc.semaphore("dve_done") as dve_sem,
        nc.semaphore("dma_out") as dma_out_sem,
        nc.semaphore("gx") as gx_sem,
        nc.Block() as block,
    ):
        PK = PK_t[0:PARTS, :]
        O = O_t[:, :]
        X21 = PK[:, 0:21]

        def ucol(m):
            return PK[:, C_U3 + m:C_U3 + m + 1]

        def hcol(m):
            return PK[:, C_H3 + m:C_H3 + m + 1]

        def _pseudo_barrier(eng):
            # NRT expands this to a real all-engine barrier on runtime
            # semaphores outside the kernel sem range — stale-state proof.
            eng.isa(
                nc.isa.Opcode.NEURON_ISA_TPB_OPCODE_PSEUDO_SYNC_BARRIER,
                {},
                struct_name="NEURON_ISA_TPB_UNKNOWN_STRUCT",
                verify=False,
            )

        @block.gpsimd
        def _(gpsimd):
            # Stale-semaphore preamble: semaphores are NOT reset between NEFF
            # executions, and waits here use absolute values.  Clear every sem
            # this kernel waits on or increments, THEN barrier — without the
            # barrier an engine can pass its first wait on a stale value
            # before the clear lands (observed as a HW deadlock).
            nums = sorted(
                x.num
                for x in (dma_in_sem, v_sem, dve_sem, dma_out_sem, gx_sem)
            )
            assert nums[-1] - nums[0] == 4, nums
            gpsimd.sem_clear(range(nums[0], nums[-1] + 1))
            # Identity gather indices, pre-compensated for the gather
            # ucode's 16-entry stream skip (HW-verified stable: SBUF
            # partition p receives index-stream entry p+16; the first 16
            # entries are consumed as pipeline prime/header).  Stream entry
            # s must therefore hold row s-16, clamped to 0 for the 16
            # discarded entries so every table value stays a valid row.
            # iota runs under the default `standard` gpsimd library; then a
            # single production-style load_library(mlp) makes DMAGatherAnt
            # available.
            gpsimd.iota(out=IDX_t[:, :], pattern=[[16, 8]], base=-16,
                        channel_multiplier=1).then_inc(gx_sem, 1)
            gpsimd.wait_ge(gx_sem, 1)
            gpsimd.tensor_scalar(out=IDX_t[:, :], in0=IDX_t[:, :],
                                 scalar1=0, scalar2=GROWS - 1,
                                 op0=Alu.max, op1=Alu.min).then_inc(gx_sem, 1)
            gpsimd.wait_ge(gx_sem, 2)
            from concourse import library_config
            gpsimd.load_library(library_config.mlp)
            gpsimd.dma_gather(
                out_ap=PK_t[:, :].rearrange("p (a e) -> p a e", a=1),
                in_ap=packed[:, :],
                idxs_ap=IDX_t[:, :],
                num_idxs=128,
                num_idxs_reg=128,
                elem_size=GCOLS,
            ).then_inc(dma_in_sem, 16)
            _pseudo_barrier(gpsimd)

        @block.scalar
        def _(scalar):
            _pseudo_barrier(scalar)

        @block.tensor
        def _(tensor):
            _pseudo_barrier(tensor)

        @block.sync
        def _(sync):
            _pseudo_barrier(sync)
            sync.wait_ge(dve_sem, 1)
            sync.dma_start(
                out=out[:, :, :].rearrange("j (g q) m -> (j g) (q m)", g=G),
                in_=O,
            ).then_inc(dma_out_sem, 16)
            sync.wait_ge(dma_out_sem, 16)

        @block.vector
        def _(vector):
            _pseudo_barrier(vector)
            vector.wait_ge(dma_in_sem, 16)

            # Every cross-op RAW dep is sem-synced: each op bumps v_sem,
            # consumers wait on the producer's cumulative count.
            def op(k, *args, **kw):
                return getattr(vector, k)(*args, **kw).then_inc(v_sem, 1)

            # ---- R numerators, one scalar_tensor_tensor per element ----
            # ops 1..9:  N9[3m+n] = u_m * u_n + W[m,n]
            for m in range(3):
                for n in range(3):
                    k = 3 * m + n
                    op("scalar_tensor_tensor", out=N9_t[:, k:k + 1],
                       in0=ucol(m), scalar=ucol(n),
                       in1=PK[:, C_W9 + k:C_W9 + k + 1],
                       op0=Alu.mult, op1=Alu.add)
            # ops 10..13: s/2 then 2/s, all single-element (near-free)
            op("tensor_scalar", out=A_t[:, :], in0=ucol(0), scalar1=hcol(0),  # 10
               scalar2=0.5, op0=Alu.mult, op1=Alu.add)         # bb/2 + 1/2
            vector.wait_ge(v_sem, 10)
            op("scalar_tensor_tensor", out=B_t[:, :], in0=ucol(1),           # 11
               scalar=hcol(1), in1=A_t[:, :], op0=Alu.mult, op1=Alu.add)
            vector.wait_ge(v_sem, 11)
            op("scalar_tensor_tensor", out=S2_t[:, :], in0=ucol(2),          # 12
               scalar=hcol(2), in1=B_t[:, :], op0=Alu.mult, op1=Alu.add)
            vector.wait_ge(v_sem, 12)
            op("reciprocal", out=IV2_t[:, :], in_=S2_t[:, :])  # 2/s         # 13
            # ---- per-point work ----
            op("tensor_tensor",                                              # 14
               out=TX_t[:, :].rearrange("p (q n) -> p q n", n=3),
               in0=PK[:, C_T:C_T + 3].unsqueeze(1).broadcast_to([PARTS, 7, 3]),
               in1=X21.rearrange("p (q n) -> p q n", n=3),
               op=Alu.subtract)                                # TX = t - X
            vector.wait_ge(v_sem, 9)
            # PA[q,n,m] = X[q,m] * N9[m,n]; one op, 3 broadcast free dims.
            op("tensor_tensor",                                              # 15
               out=PA_t[:, :].rearrange("p (q n m) -> p q n m", n=3, m=3),
               in0=PK_t[0:PARTS, 0:21].rearrange("p (q m) -> p q m", m=3)
                   .unsqueeze(2).broadcast_to([PARTS, 7, 3, 3]),
                                                               # X: (q s3,n s0,m s1)
               in1=N9_t[:, 0:9].rearrange("p (m n) -> p n m", n=3)
                   .unsqueeze(1).broadcast_to([PARTS, 7, 3, 3]),
                                                               # N9: (q s0,n s1,m s3)
               op=Alu.mult)
            vector.wait_ge(v_sem, 15)
            op("reduce_sum", out=ZN_t[:, :],                                 # 16
               in_=PA_t[:, :].rearrange("p (q n m) -> p q n m", n=3, m=3),
               axis=mybir.AxisListType.X)                      # sum over m
            vector.wait_ge(v_sem, 16)
            vector.scalar_tensor_tensor(                                     # 17
                out=O, in0=ZN_t[:, :], scalar=IV2_t[:, :], in1=TX_t[:, :],
                op0=Alu.mult, op1=Alu.add,                     # ZN*2/s + (t-X)
            ).then_inc(dve_sem, 1)

    return nc


def get_nc():
    if "nc" not in _cache:
        nc = _build_nc()
        # Raw Bass skips Bacc's codegen pass that fills in .instr bytes for
        # extended InstISA subclasses (the library reload); without it the
        # NEFF compiler fails with "ISA wrong length".
        from concourse.library_overlay import lower_extended_insts

        lower_extended_insts(nc)
        _cache["nc"] = nc
    return _cache["nc"]


def shard_inputs(pred_coor, r_vector, t_vector):
    n = pred_coor.shape[0]
    b, c, d = r_vector[:, 0], r_vector[:, 1], r_vector[:, 2]
    one = np.ones_like(b)
    w9 = np.stack([one, -d, c, d, one, -b, -c, b, one], axis=-1)  # [n,9]
    pk = np.empty((n, G, NPACK), dtype=np.float32)
    pk[:, :, 0:21] = pred_coor.reshape(n, G, 21)
    pk[:, :, C_U3:C_U3 + 3] = r_vector[:, None, :]
    pk[:, :, C_W9:C_W9 + 9] = w9[:, None, :]
    pk[:, :, C_T:C_T + 3] = t_vector[:, None, :]
    pk[:, :, C_H3:C_H3 + 3] = 0.5 * r_vector[:, None, :]
    pk = pk.reshape(n * G, NPACK)
    out_maps = []
    for c in range(NCORES):
        g = np.zeros((GROWS, GCOLS), dtype=np.float32)
        g[:PARTS, :NPACK] = pk[c * PARTS : (c + 1) * PARTS]
        out_maps.append({"packed": g})
    return out_maps


def run(pred_coor, r_vector, t_vector, trace=False):
    from concourse.bass_utils import run_bass_kernel_spmd

    nc = get_nc()
    in_maps = shard_inputs(pred_coor, r_vector, t_vector)
    res = run_bass_kernel_spmd(nc, in_maps, list(range(NCORES)), trace=trace)
    full = np.concatenate([res.results[c]["out"] for c in range(NCORES)], axis=0)
    return full, res


def kernel(pred_coor, r_vector, t_vector):
    pred_coor = np.asarray(pred_coor, dtype=np.float32)
    r_vector = np.asarray(r_vector, dtype=np.float32)
    t_vector = np.asarray(t_vector, dtype=np.float32)
    full, _ = run(pred_coor, r_vector, t_vector, trace=False)
    return full
